# revision 3
# baseline (speedup 1.0000x reference)
"""Trainium2 Bass kernel for GQA attention layer (B=1, T=2048, HID=4096,
32 q-heads / 8 kv-heads, head_dim 128, RoPE, causal) sharded over 8 cores.

Sharding: tensor-parallel over heads. Core c owns q-heads 4c..4c+3 and
kv-head c. After attention, the per-core attention outputs (transposed,
[512 hd, 2048 t]) are AllGathered; each core then computes a 512-row
slice of the output projection (contraction over the full 4096 hd dims),
so no AllReduce is needed. Host assembles the 8 output slices.

All matmuls run in float32r (fp32 with 11-bit mantissa, full PE rate at
moving-dim >= 256), accumulation in fp32 PSUM.
"""

import numpy as np

import concourse.bacc as bacc
import concourse.mybir as mybir
import concourse.tile as tile
from concourse.bass_utils import run_bass_kernel_spmd

T = 2048
HID = 4096
D = 128
N_HEADS = 32
N_KV = 8
HQ = N_HEADS // N_KV  # q heads per core (=4)
TT = 512  # t tile
NTT = T // TT  # 4
NH = HID // 128  # 32 h-tiles
SCALE = 1.0 / np.sqrt(np.float32(D))
ROPE_BASE = 10000.0
N_CORES = 8

_F32 = mybir.dt.float32
_DT = mybir.dt.float32r

_cached = None


def _build():
    nc = bacc.Bacc("TRN2", target_bir_lowering=False, debug=False, num_devices=N_CORES)

    xT = nc.dram_tensor("xT", [HID, T], _DT, kind="ExternalInput").ap()
    wqT = nc.dram_tensor("wqT", [HID, HQ * D], _DT, kind="ExternalInput").ap()
    wkT = nc.dram_tensor("wkT", [HID, D], _DT, kind="ExternalInput").ap()
    wvT = nc.dram_tensor("wvT", [HID, D], _DT, kind="ExternalInput").ap()
    woT = nc.dram_tensor("woT", [HID, HQ * D], _DT, kind="ExternalInput").ap()
    cos2 = nc.dram_tensor("cos2", [128, T], _DT, kind="ExternalInput").ap()
    sinS = nc.dram_tensor("sinS", [128, T], _DT, kind="ExternalInput").ap()
    masks = nc.dram_tensor("masks", [128, 4 * TT], _DT, kind="ExternalInput").ap()
    ones_i = nc.dram_tensor("ones_i", [128, 128], _DT, kind="ExternalInput").ap()
    ident_i = nc.dram_tensor("ident_i", [128, 128], _DT, kind="ExternalInput").ap()
    out = nc.dram_tensor("out", [HQ * D, T], _F32, kind="ExternalOutput").ap()

    Exp = mybir.ActivationFunctionType.Exp

    with tile.TileContext(nc) as tc:
        with (
            tc.tile_pool(name="const", bufs=1) as const,
            tc.tile_pool(name="big", bufs=1) as big,
            tc.tile_pool(name="sb", bufs=1) as sb,
            tc.tile_pool(name="ps", bufs=1, space="PSUM") as ps,
            tc.tile_pool(name="dram", bufs=1, space="DRAM") as dram,
        ):
            # ---- constants / persistent tiles ----
            wk_sb = const.tile([128, HID], _DT, name="wk_sb")
            wv_sb = const.tile([128, HID], _DT, name="wv_sb")
            for j in range(NH):
                nc.sync.dma_start(
                    out=wk_sb[:, 128 * j : 128 * (j + 1)],
                    in_=wkT[128 * j : 128 * (j + 1), :],
                )
                nc.sync.dma_start(
                    out=wv_sb[:, 128 * j : 128 * (j + 1)],
                    in_=wvT[128 * j : 128 * (j + 1), :],
                )
            cos_sb = const.tile([128, T], _DT, name="cos_sb")
            sin_sb = const.tile([128, T], _DT, name="sin_sb")
            mask_sb = const.tile([128, 4 * TT], _DT, name="mask_sb")
            ones_sb = const.tile([128, 128], _DT, name="ones_sb")
            ident_sb = const.tile([128, 128], _DT, name="ident_sb")
            nc.sync.dma_start(out=cos_sb[:], in_=cos2[:])
            nc.sync.dma_start(out=sin_sb[:], in_=sinS[:])
            nc.sync.dma_start(out=mask_sb[:], in_=masks[:])
            nc.sync.dma_start(out=ones_sb[:], in_=ones_i[:])
            nc.sync.dma_start(out=ident_sb[:], in_=ident_i[:])

            qrot = [big.tile([128, T], _DT, name=f"qrot{h}") for h in range(HQ)]
            krot = big.tile([128, T], _DT, name="krot")
            v_sb = big.tile([128, T], _DT, name="v_sb")  # V[s,d]: block k at cols 128k

            attn_local = dram.tile([HQ * D, T], _DT, name="attn_local")
            attn_full = dram.tile(
                [N_CORES * HQ * D, T], _DT, addr_space="Shared", name="attn_full"
            )

            for ti in range(NTT):
                tsl = slice(TT * ti, TT * (ti + 1))
                # ---- q/k/v projections for this t tile ----
                q_ps = [ps.tile([128, TT], _F32, tag=f"p{h}", name=f"q_ps{h}") for h in range(HQ)]
                k_ps = ps.tile([128, TT], _F32, tag="p4")
                vT_ps = ps.tile([128, TT], _F32, tag="p5")
                for hi in range(NH):
                    hsl = slice(128 * hi, 128 * (hi + 1))
                    xt = sb.tile([128, TT], _DT, tag="x", bufs=4)
                    nc.sync.dma_start(out=xt[:], in_=xT[hsl, tsl])
                    wqt = sb.tile([128, HQ * D], _DT, tag="wq", bufs=4)
                    nc.sync.dma_start(out=wqt[:], in_=wqT[hsl, :])
                    st, sp = hi == 0, hi == NH - 1
                    for h in range(HQ):
                        nc.tensor.matmul(
                            q_ps[h][:],
                            wqt[:, 128 * h : 128 * (h + 1)],
                            xt[:],
                            start=st,
                            stop=sp,
                        )
                    nc.tensor.matmul(k_ps[:], wk_sb[:, hsl], xt[:], start=st, stop=sp)
                    nc.tensor.matmul(vT_ps[:], wv_sb[:, hsl], xt[:], start=st, stop=sp)

                # ---- V: transpose [d, s] -> [s, d] blocks ----
                vT_sb = sb.tile([128, TT], _DT, tag="vTs", bufs=2)
                nc.vector.tensor_copy(vT_sb[:], vT_ps[:])
                for j in range(TT // 128):
                    vtr = ps.tile([128, 128], _DT, tag="p6", bufs=2)
                    nc.tensor.transpose(
                        vtr[:], vT_sb[:, 128 * j : 128 * (j + 1)], ident_sb[:]
                    )
                    k = (TT // 128) * ti + j
                    nc.vector.tensor_copy(v_sb[:, 128 * k : 128 * (k + 1)], vtr[:])

                # ---- RoPE on q heads and k ----
                for h in range(HQ + 1):
                    src = q_ps[h] if h < HQ else k_ps
                    qf = sb.tile([128, TT], _DT, tag="qf", bufs=2)
                    nc.vector.tensor_copy(qf[:], src[:])
                    qs = sb.tile([128, TT], _DT, tag="qs", bufs=2)
                    nc.sync.dma_start(out=qs[0:64, :], in_=qf[64:128, :])
                    nc.sync.dma_start(out=qs[64:128, :], in_=qf[0:64, :])
                    t1 = sb.tile([128, TT], _F32, tag="t1", bufs=2)
                    nc.vector.tensor_mul(t1[:], qf[:], cos_sb[:, tsl])
                    t2 = sb.tile([128, TT], _F32, tag="t2", bufs=2)
                    nc.vector.tensor_mul(t2[:], qs[:], sin_sb[:, tsl])
                    dst = qrot[h][:, tsl] if h < HQ else krot[:, tsl]
                    nc.vector.tensor_add(dst, t1[:], t2[:])

                # ---- causal attention for this t tile ----
                nblk = (TT // 128) * (ti + 1)
                for h in range(HQ):
                    attn_ps = ps.tile([128, TT], _F32, tag=f"p{h}")
                    den_ps = ps.tile([128, TT], _F32, tag="p4" if h % 2 == 0 else "p5")
                    for k in range(nblk):
                        ksl = slice(128 * k, 128 * (k + 1))
                        sc = ps.tile([128, TT], _F32, tag="p6", bufs=2)
                        nc.tensor.matmul(
                            sc[:], krot[:, ksl], qrot[h][:, tsl], start=True, stop=True
                        )
                        st, sp = k == 0, k == nblk - 1
                        diag = k - (TT // 128) * ti
                        if diag >= 0:
                            ptmp = sb.tile([128, TT], _F32, tag="ptmp", bufs=2)
                            nc.scalar.activation(ptmp[:], sc[:], Exp, scale=SCALE)
                            probs = sb.tile([128, TT], _DT, tag="probs", bufs=3)
                            nc.vector.tensor_mul(
                                probs[:],
                                ptmp[:],
                                mask_sb[:, TT * diag : TT * (diag + 1)],
                            )
                        else:
                            probs = sb.tile([128, TT], _DT, tag="probs", bufs=3)
                            nc.scalar.activation(probs[:], sc[:], Exp, scale=SCALE)
                        nc.tensor.matmul(
                            attn_ps[:], v_sb[:, ksl], probs[:], start=st, stop=sp
                        )
                        nc.tensor.matmul(
                            den_ps[:], ones_sb[:], probs[:], start=st, stop=sp
                        )
                    recip = sb.tile([128, TT], _F32, tag="recip", bufs=2)
                    nc.vector.reciprocal(recip[:], den_ps[:])
                    anorm = sb.tile([128, TT], _DT, tag="anorm", bufs=2)
                    nc.vector.tensor_mul(anorm[:], attn_ps[:], recip[:])
                    nc.sync.dma_start(
                        out=attn_local[128 * h : 128 * (h + 1), tsl], in_=anorm[:]
                    )

            # ---- gather attention outputs from all cores ----
            nc.gpsimd.collective_compute(
                "AllGather",
                mybir.AluOpType.bypass,
                replica_groups=[list(range(N_CORES))],
                ins=[attn_local.opt()],
                outs=[attn_full.opt()],
            )

            # ---- output projection: out[o_local, t] over full 4096 hd ----
            TH = 1024  # t half
            for th in range(2):
                o_ps = []
                for i in range(8):
                    tag = f"p{i}" if i < 6 else "p6"
                    o_ps.append(
                        ps.tile(
                            [128, TT],
                            _F32,
                            tag=tag,
                            name=f"o_ps{i}",
                            bufs=1 if i < 6 else 2,
                        )
                    )
                for hd in range(HID // 128):
                    hsl = slice(128 * hd, 128 * (hd + 1))
                    ag = sb.tile([128, TH], _DT, tag="ag", bufs=4)
                    nc.sync.dma_start(
                        out=ag[:], in_=attn_full[hsl, TH * th : TH * (th + 1)]
                    )
                    wo = sb.tile([128, HQ * D], _DT, tag="wo", bufs=4)
                    nc.sync.dma_start(out=wo[:], in_=woT[hsl, :])
                    st, sp = hd == 0, hd == HID // 128 - 1
                    for o in range(4):
                        for tt in range(2):
                            nc.tensor.matmul(
                                o_ps[o * 2 + tt][:],
                                wo[:, 128 * o : 128 * (o + 1)],
                                ag[:, TT * tt : TT * (tt + 1)],
                                start=st,
                                stop=sp,
                            )
                for o in range(4):
                    for tt in range(2):
                        oc = sb.tile([128, TT], _F32, tag="oc", bufs=3)
                        nc.vector.tensor_copy(oc[:], o_ps[o * 2 + tt][:])
                        nc.sync.dma_start(
                            out=out[
                                128 * o : 128 * (o + 1),
                                TH * th + TT * tt : TH * th + TT * (tt + 1),
                            ],
                            in_=oc[:],
                        )

    nc.compile()
    return nc


def _host_inputs(hidden_states, Wq, Wk, Wv, Wo):
    x = np.asarray(hidden_states, dtype=np.float32).reshape(T, HID)
    xT = np.ascontiguousarray(x.T)

    pos = np.arange(T, dtype=np.float32)
    inv_freq = ROPE_BASE ** (-np.arange(0, D, 2, dtype=np.float32) / D)  # [64]
    ang = pos[:, None] * inv_freq[None, :]  # [T, 64]
    cosT = np.cos(ang).T.astype(np.float32)  # [64, T]
    sinT = np.sin(ang).T.astype(np.float32)
    cos2 = np.ascontiguousarray(np.concatenate([cosT, cosT], axis=0))
    sinS = np.ascontiguousarray(np.concatenate([-sinT, sinT], axis=0))

    p = np.arange(128)[:, None]
    tp = np.arange(TT)[None, :]
    masks = np.concatenate(
        [(p <= tp - 128 * j).astype(np.float32) for j in range(4)], axis=1
    )
    masks = np.ascontiguousarray(masks)
    ones = np.ones((128, 128), dtype=np.float32)
    ident = np.eye(128, dtype=np.float32)

    Wq = np.asarray(Wq, dtype=np.float32)
    Wk = np.asarray(Wk, dtype=np.float32)
    Wv = np.asarray(Wv, dtype=np.float32)
    Wo = np.asarray(Wo, dtype=np.float32)

    in_maps = []
    for c in range(N_CORES):
        qs = slice(HQ * D * c, HQ * D * (c + 1))
        ks = slice(D * c, D * (c + 1))
        in_maps.append(
            {
                "xT": xT,
                "wqT": np.ascontiguousarray(Wq[qs, :].T),
                "wkT": np.ascontiguousarray(Wk[ks, :].T),
                "wvT": np.ascontiguousarray(Wv[ks, :].T),
                "woT": np.ascontiguousarray(Wo[qs, :].T),
                "cos2": cos2,
                "sinS": sinS,
                "masks": masks,
                "ones_i": ones,
                "ident_i": ident,
            }
        )
    return in_maps


def get_program():
    global _cached
    if _cached is None:
        _cached = _build()
    return _cached


def kernel(hidden_states, Wq, Wk, Wv, Wo):
    nc = get_program()
    in_maps = _host_inputs(hidden_states, Wq, Wk, Wv, Wo)
    res = run_bass_kernel_spmd(nc, in_maps, list(range(N_CORES)))
    outT = np.concatenate([res.results[c]["out"] for c in range(N_CORES)], axis=0)
    return np.ascontiguousarray(outT.T).reshape(1, T, HID).astype(np.float32)


# revision 5
# speedup vs baseline: 1.4917x; 1.4917x over previous
"""Trainium2 Bass kernel for GQA attention layer (B=1, T=2048, HID=4096,
32 q-heads / 8 kv-heads, head_dim 128, RoPE, causal) sharded over 8 cores.

Sharding: tensor-parallel over heads. Core c owns q-heads 4c..4c+3 and
kv-head c. Attention outputs (transposed, [512 hd, t]) are AllGathered in
four t-chunks (pipelined against later attention compute); each core then
computes a 512-row slice of the output projection over the full 4096 hd
dims, so no AllReduce is needed. Host assembles the 8 output slices.

Matmuls run in bf16 (PE moving operand streams 2B/cycle, so bf16 is 2x
fp32r), accumulation in fp32 PSUM; softmax statistics in fp32.
"""

import numpy as np

import concourse.bacc as bacc
import concourse.mybir as mybir
import concourse.tile as tile
from concourse.bass_utils import run_bass_kernel_spmd

T = 2048
HID = 4096
D = 128
N_HEADS = 32
N_KV = 8
HQ = N_HEADS // N_KV  # q heads per core (=4)
TT = 512  # t tile
NTT = T // TT  # 4
NH = HID // 128  # 32 h-tiles
SCALE = 1.0 / np.sqrt(np.float32(D))
ROPE_BASE = 10000.0
N_CORES = 8

_F32 = mybir.dt.float32
_DT = mybir.dt.bfloat16

_cached = None


def _build():
    nc = bacc.Bacc("TRN2", target_bir_lowering=False, debug=False, num_devices=N_CORES)

    xT = nc.dram_tensor("xT", [HID, T], _DT, kind="ExternalInput").ap()
    wqT = nc.dram_tensor("wqT", [HID, HQ * D], _DT, kind="ExternalInput").ap()
    wkT = nc.dram_tensor("wkT", [HID, D], _DT, kind="ExternalInput").ap()
    wvT = nc.dram_tensor("wvT", [HID, D], _DT, kind="ExternalInput").ap()
    woT = nc.dram_tensor("woT", [HID, HQ * D], _DT, kind="ExternalInput").ap()
    cos2 = nc.dram_tensor("cos2", [128, T], _F32, kind="ExternalInput").ap()
    sinS = nc.dram_tensor("sinS", [128, T], _F32, kind="ExternalInput").ap()
    masks = nc.dram_tensor("masks", [128, 4 * TT], _F32, kind="ExternalInput").ap()
    ones_i = nc.dram_tensor("ones_i", [128, 128], _DT, kind="ExternalInput").ap()
    ident_i = nc.dram_tensor("ident_i", [128, 128], _DT, kind="ExternalInput").ap()
    out = nc.dram_tensor("out", [HQ * D, T], _F32, kind="ExternalOutput").ap()

    Exp = mybir.ActivationFunctionType.Exp

    with tile.TileContext(nc) as tc:
        with (
            tc.tile_pool(name="const", bufs=1) as const,
            tc.tile_pool(name="big", bufs=1) as big,
            tc.tile_pool(name="sb", bufs=1) as sb,
            tc.tile_pool(name="ps", bufs=1, space="PSUM") as ps,
            tc.tile_pool(name="dram", bufs=1, space="DRAM") as dram,
        ):
            # ---- constants / persistent weights in SBUF ----
            cos_sb = const.tile([128, T], _F32, name="cos_sb")
            sin_sb = const.tile([128, T], _F32, name="sin_sb")
            mask_sb = const.tile([128, 4 * TT], _F32, name="mask_sb")
            ones_sb = const.tile([128, 128], _DT, name="ones_sb")
            ident_sb = const.tile([128, 128], _DT, name="ident_sb")
            # per-h-tile weight tiles so the first matmuls only wait on
            # their own slice's DMA, not the whole preload
            wq_t = [const.tile([128, HQ * D], _DT, name=f"wq_t{j}") for j in range(NH)]
            wk_t = [const.tile([128, D], _DT, name=f"wk_t{j}") for j in range(NH)]
            wv_t = [const.tile([128, D], _DT, name=f"wv_t{j}") for j in range(NH)]
            wo_sb = const.tile([128, NH * HQ * D], _DT, name="wo_sb")
            for j in range(NH):
                hsl = slice(128 * j, 128 * (j + 1))
                nc.sync.dma_start(out=wq_t[j][:], in_=wqT[hsl, :])
                nc.sync.dma_start(out=wk_t[j][:], in_=wkT[hsl, :])
                nc.sync.dma_start(out=wv_t[j][:], in_=wvT[hsl, :])
                nc.gpsimd.dma_start(
                    out=wo_sb[:, 512 * j : 512 * (j + 1)], in_=woT[hsl, :]
                )

            nc.scalar.dma_start(out=cos_sb[:], in_=cos2[:])
            nc.scalar.dma_start(out=sin_sb[:], in_=sinS[:])
            nc.scalar.dma_start(out=mask_sb[:], in_=masks[:])
            nc.scalar.dma_start(out=ones_sb[:], in_=ones_i[:])
            nc.scalar.dma_start(out=ident_sb[:], in_=ident_i[:])

            qrot = [big.tile([128, T], _DT, name=f"qrot{h}") for h in range(HQ)]
            krot = big.tile([128, T], _DT, name="krot")
            v_sb = big.tile([128, T], _DT, name="v_sb")  # V[s,d]: block k at cols 128k

            attn_local = [
                dram.tile([HQ * D, TT], _DT, name=f"attn_local{i}") for i in range(NTT)
            ]
            attn_full = [
                dram.tile(
                    [N_CORES * HQ * D, TT],
                    _DT,
                    addr_space="Shared",
                    name=f"attn_full{i}",
                )
                for i in range(NTT)
            ]

            def proj(ti):
                tsl = slice(TT * ti, TT * (ti + 1))
                q_ps = [
                    ps.tile([128, TT], _F32, tag=f"p{h}", name=f"q_ps{h}")
                    for h in range(HQ)
                ]
                k_ps = ps.tile([128, TT], _F32, tag="p4")
                vT_ps = ps.tile([128, TT], _F32, tag="p5")
                for hi in range(NH):
                    hsl = slice(128 * hi, 128 * (hi + 1))
                    xt = sb.tile([128, TT], _DT, tag="x", bufs=6)
                    nc.sync.dma_start(out=xt[:], in_=xT[hsl, tsl])
                    st, sp = hi == 0, hi == NH - 1
                    for h in range(HQ):
                        nc.tensor.matmul(
                            q_ps[h][:],
                            wq_t[hi][:, 128 * h : 128 * (h + 1)],
                            xt[:],
                            start=st,
                            stop=sp,
                        )
                    nc.tensor.matmul(k_ps[:], wk_t[hi][:], xt[:], start=st, stop=sp)
                    nc.tensor.matmul(vT_ps[:], wv_t[hi][:], xt[:], start=st, stop=sp)

                # V: transpose [d, s] -> [s, d] blocks
                vT_sb = sb.tile([128, TT], _DT, tag="vTs", bufs=2)
                nc.vector.tensor_copy(vT_sb[:], vT_ps[:])
                for j in range(TT // 128):
                    vtr = ps.tile([128, 128], _DT, tag="p6", bufs=2)
                    nc.tensor.transpose(
                        vtr[:], vT_sb[:, 128 * j : 128 * (j + 1)], ident_sb[:]
                    )
                    k = (TT // 128) * ti + j
                    nc.vector.tensor_copy(v_sb[:, 128 * k : 128 * (k + 1)], vtr[:])

                # RoPE on q heads and k (fp32 math, bf16 store)
                for h in range(HQ + 1):
                    src = q_ps[h] if h < HQ else k_ps
                    qf = sb.tile([128, TT], _F32, tag="qf", bufs=2)
                    nc.vector.tensor_copy(qf[:], src[:])
                    qs = sb.tile([128, TT], _F32, tag="qs", bufs=2)
                    nc.scalar.dma_start(out=qs[0:64, :], in_=qf[64:128, :])
                    nc.scalar.dma_start(out=qs[64:128, :], in_=qf[0:64, :])
                    t1 = sb.tile([128, TT], _F32, tag="t1", bufs=2)
                    nc.vector.tensor_mul(t1[:], qf[:], cos_sb[:, tsl])
                    t2 = sb.tile([128, TT], _F32, tag="t2", bufs=2)
                    nc.vector.tensor_mul(t2[:], qs[:], sin_sb[:, tsl])
                    dst = qrot[h][:, tsl] if h < HQ else krot[:, tsl]
                    nc.vector.tensor_add(dst, t1[:], t2[:])

            def attn(ti):
                tsl = slice(TT * ti, TT * (ti + 1))
                nblk = (TT // 128) * (ti + 1)
                for h in range(HQ):
                    attn_ps = ps.tile([128, TT], _F32, tag=f"p{h}")
                    den_ps = ps.tile([128, TT], _F32, tag="p4" if h % 2 == 0 else "p5")
                    for k in range(nblk):
                        ksl = slice(128 * k, 128 * (k + 1))
                        st, sp = k == 0, k == nblk - 1
                        diag = k - (TT // 128) * ti
                        # causal: block k only reaches t' >= 128*diag
                        lo = 128 * diag if diag > 0 else 0
                        qsl = slice(TT * ti + lo, TT * (ti + 1))
                        sc = ps.tile([128, TT], _F32, tag="p6", bufs=2)
                        nc.tensor.matmul(
                            sc[:, lo:TT],
                            krot[:, ksl],
                            qrot[h][:, qsl],
                            start=True,
                            stop=True,
                        )
                        probs = sb.tile([128, TT], _DT, tag="probs", bufs=3)
                        if diag >= 0:
                            ptmp = sb.tile([128, TT], _F32, tag="ptmp", bufs=2)
                            nc.scalar.activation(
                                ptmp[:, lo:TT], sc[:, lo:TT], Exp, scale=SCALE
                            )
                            nc.vector.tensor_mul(
                                probs[:, lo:TT],
                                ptmp[:, lo:TT],
                                mask_sb[:, TT * diag + lo : TT * (diag + 1)],
                            )
                        else:
                            nc.scalar.activation(probs[:], sc[:], Exp, scale=SCALE)
                        nc.tensor.matmul(
                            attn_ps[:, lo:TT],
                            v_sb[:, ksl],
                            probs[:, lo:TT],
                            start=st,
                            stop=sp,
                        )
                        nc.tensor.matmul(
                            den_ps[:, lo:TT],
                            ones_sb[:],
                            probs[:, lo:TT],
                            start=st,
                            stop=sp,
                        )
                    recip = sb.tile([128, TT], _F32, tag="recip", bufs=2)
                    nc.vector.reciprocal_approx_fast(recip[:], den_ps[:])
                    anorm = sb.tile([128, TT], _DT, tag="anorm", bufs=2)
                    nc.vector.tensor_mul(anorm[:], attn_ps[:], recip[:])
                    nc.gpsimd.dma_start(
                        out=attn_local[ti][128 * h : 128 * (h + 1), :], in_=anorm[:]
                    )

            def gather(ti):
                nc.gpsimd.collective_compute(
                    "AllGather",
                    mybir.AluOpType.bypass,
                    replica_groups=[list(range(N_CORES))],
                    ins=[attn_local[ti].opt()],
                    outs=[attn_full[ti].opt()],
                )

            def outproj(ti, tags):
                o_ps = [
                    ps.tile(
                        [128, TT],
                        _F32,
                        tag=tg,
                        name=f"o_ps{ti}_{i}",
                        bufs=2 if tg == "p6" else 1,
                    )
                    for i, tg in enumerate(tags)
                ]
                for hd in range(NH):
                    ag = sb.tile([128, TT], _DT, tag="ag", bufs=6)
                    nc.gpsimd.dma_start(
                        out=ag[:], in_=attn_full[ti][128 * hd : 128 * (hd + 1), :]
                    )
                    st, sp = hd == 0, hd == NH - 1
                    for o in range(4):
                        nc.tensor.matmul(
                            o_ps[o][:],
                            wo_sb[:, 512 * hd + 128 * o : 512 * hd + 128 * (o + 1)],
                            ag[:],
                            start=st,
                            stop=sp,
                        )
                for o in range(4):
                    oc = sb.tile([128, TT], _F32, tag="oc", bufs=4)
                    nc.vector.tensor_copy(oc[:], o_ps[o][:])
                    nc.sync.dma_start(
                        out=out[128 * o : 128 * (o + 1), TT * ti : TT * (ti + 1)],
                        in_=oc[:],
                    )

            # pipeline: AG(ti) overlaps attn(ti+1); outproj(ti) follows
            proj(0)
            attn(0)
            gather(0)
            proj(1)
            attn(1)
            gather(1)
            outproj(0, ["p0", "p1", "p2", "p3"])
            proj(2)
            attn(2)
            gather(2)
            outproj(1, ["p0", "p1", "p2", "p3"])
            proj(3)
            attn(3)
            gather(3)
            outproj(2, ["p0", "p1", "p2", "p3"])
            outproj(3, ["p4", "p5", "p6", "p6"])

    nc.compile()
    return nc


def _host_inputs(hidden_states, Wq, Wk, Wv, Wo):
    import ml_dtypes

    bf16 = ml_dtypes.bfloat16
    x = np.asarray(hidden_states, dtype=np.float32).reshape(T, HID)
    xT = np.ascontiguousarray(x.T).astype(bf16)

    pos = np.arange(T, dtype=np.float32)
    inv_freq = ROPE_BASE ** (-np.arange(0, D, 2, dtype=np.float32) / D)  # [64]
    ang = pos[:, None] * inv_freq[None, :]  # [T, 64]
    cosT = np.cos(ang).T.astype(np.float32)  # [64, T]
    sinT = np.sin(ang).T.astype(np.float32)
    cos2 = np.ascontiguousarray(np.concatenate([cosT, cosT], axis=0))
    sinS = np.ascontiguousarray(np.concatenate([-sinT, sinT], axis=0))

    p = np.arange(128)[:, None]
    tp = np.arange(TT)[None, :]
    masks = np.concatenate(
        [(p <= tp - 128 * j).astype(np.float32) for j in range(4)], axis=1
    )
    masks = np.ascontiguousarray(masks)
    ones = np.ones((128, 128), dtype=bf16)
    ident = np.eye(128, dtype=np.float32).astype(bf16)

    Wq = np.asarray(Wq, dtype=np.float32)
    Wk = np.asarray(Wk, dtype=np.float32)
    Wv = np.asarray(Wv, dtype=np.float32)
    Wo = np.asarray(Wo, dtype=np.float32)

    in_maps = []
    for c in range(N_CORES):
        qs = slice(HQ * D * c, HQ * D * (c + 1))
        ks = slice(D * c, D * (c + 1))
        in_maps.append(
            {
                "xT": xT,
                "wqT": np.ascontiguousarray(Wq[qs, :].T).astype(bf16),
                "wkT": np.ascontiguousarray(Wk[ks, :].T).astype(bf16),
                "wvT": np.ascontiguousarray(Wv[ks, :].T).astype(bf16),
                "woT": np.ascontiguousarray(Wo[qs, :].T).astype(bf16),
                "cos2": cos2,
                "sinS": sinS,
                "masks": masks,
                "ones_i": ones,
                "ident_i": ident,
            }
        )
    return in_maps


def get_program():
    global _cached
    if _cached is None:
        _cached = _build()
    return _cached


def kernel(hidden_states, Wq, Wk, Wv, Wo):
    nc = get_program()
    in_maps = _host_inputs(hidden_states, Wq, Wk, Wv, Wo)
    res = run_bass_kernel_spmd(nc, in_maps, list(range(N_CORES)))
    outT = np.concatenate([res.results[c]["out"] for c in range(N_CORES)], axis=0)
    return np.ascontiguousarray(outT.T).reshape(1, T, HID).astype(np.float32)


# revision 6
# speedup vs baseline: 1.5587x; 1.0449x over previous
"""Trainium2 Bass kernel for GQA attention layer (B=1, T=2048, HID=4096,
32 q-heads / 8 kv-heads, head_dim 128, RoPE, causal) sharded over 8 cores.

Sharding: tensor-parallel over heads. Core c owns q-heads 4c..4c+3 and
kv-head c. Attention outputs (transposed, [512 hd, t]) are AllGathered in
four t-chunks (pipelined against later attention compute); each core then
computes a 512-row slice of the output projection over the full 4096 hd
dims, so no AllReduce is needed. Host assembles the 8 output slices.

Matmuls run in bf16 (PE moving operand streams 2B/cycle, so bf16 is 2x
fp32r), accumulation in fp32 PSUM; softmax statistics in fp32.
"""

import numpy as np

import concourse.bacc as bacc
import concourse.mybir as mybir
import concourse.tile as tile
from concourse.bass_utils import run_bass_kernel_spmd

T = 2048
HID = 4096
D = 128
N_HEADS = 32
N_KV = 8
HQ = N_HEADS // N_KV  # q heads per core (=4)
TT = 512  # t tile
NTT = T // TT  # 4
NH = HID // 128  # 32 h-tiles
SCALE = 1.0 / np.sqrt(np.float32(D))
ROPE_BASE = 10000.0
N_CORES = 8

_F32 = mybir.dt.float32
_DT = mybir.dt.bfloat16

_cached = None


def _build():
    nc = bacc.Bacc("TRN2", target_bir_lowering=False, debug=False, num_devices=N_CORES)

    xT = nc.dram_tensor("xT", [HID, T], _DT, kind="ExternalInput").ap()
    wqT = nc.dram_tensor("wqT", [HID, HQ * D], _DT, kind="ExternalInput").ap()
    wkT = nc.dram_tensor("wkT", [HID, D], _DT, kind="ExternalInput").ap()
    wvT = nc.dram_tensor("wvT", [HID, D], _DT, kind="ExternalInput").ap()
    woT = nc.dram_tensor("woT", [HID, HQ * D], _DT, kind="ExternalInput").ap()
    cos2 = nc.dram_tensor("cos2", [128, T], _F32, kind="ExternalInput").ap()
    sinS = nc.dram_tensor("sinS", [128, T], _F32, kind="ExternalInput").ap()
    masks = nc.dram_tensor("masks", [128, 4 * TT], _F32, kind="ExternalInput").ap()
    ones_i = nc.dram_tensor("ones_i", [128, 128], _DT, kind="ExternalInput").ap()
    ident_i = nc.dram_tensor("ident_i", [128, 128], _DT, kind="ExternalInput").ap()
    out = nc.dram_tensor("out", [HQ * D, T], _F32, kind="ExternalOutput").ap()

    Exp = mybir.ActivationFunctionType.Exp

    with tile.TileContext(nc) as tc:
        with (
            tc.tile_pool(name="const", bufs=1) as const,
            tc.tile_pool(name="big", bufs=1) as big,
            tc.tile_pool(name="sb", bufs=1) as sb,
            tc.tile_pool(name="ps", bufs=1, space="PSUM") as ps,
            tc.tile_pool(name="dram", bufs=1, space="DRAM") as dram,
        ):
            # ---- constants / persistent weights in SBUF ----
            cos_sb = const.tile([128, T], _F32, name="cos_sb")
            sin_sb = const.tile([128, T], _F32, name="sin_sb")
            mask_sb = const.tile([128, 4 * TT], _F32, name="mask_sb")
            ones_sb = const.tile([128, 128], _DT, name="ones_sb")
            ident_sb = const.tile([128, 128], _DT, name="ident_sb")
            # per-h-tile weight tiles so the first matmuls only wait on
            # their own slice's DMA, not the whole preload
            wq_t = [const.tile([128, HQ * D], _DT, name=f"wq_t{j}") for j in range(NH)]
            wk_t = [const.tile([128, D], _DT, name=f"wk_t{j}") for j in range(NH)]
            wv_t = [const.tile([128, D], _DT, name=f"wv_t{j}") for j in range(NH)]
            wo_sb = const.tile([128, NH * HQ * D], _DT, name="wo_sb")
            for j in range(NH):
                hsl = slice(128 * j, 128 * (j + 1))
                nc.gpsimd.dma_start(
                    out=wo_sb[:, 512 * j : 512 * (j + 1)], in_=woT[hsl, :]
                )

            nc.scalar.dma_start(out=cos_sb[:], in_=cos2[:])
            nc.scalar.dma_start(out=sin_sb[:], in_=sinS[:])
            nc.scalar.dma_start(out=mask_sb[:], in_=masks[:])
            nc.scalar.dma_start(out=ones_sb[:], in_=ones_i[:])
            nc.scalar.dma_start(out=ident_sb[:], in_=ident_i[:])

            qrot = [big.tile([128, T], _DT, name=f"qrot{h}") for h in range(HQ)]
            krot = big.tile([128, T], _DT, name="krot")
            v_sb = big.tile([128, T], _DT, name="v_sb")  # V[s,d]: block k at cols 128k

            attn_local = [
                dram.tile([HQ * D, TT], _DT, name=f"attn_local{i}") for i in range(NTT)
            ]
            attn_full = [
                dram.tile(
                    [N_CORES * HQ * D, TT],
                    _DT,
                    addr_space="Shared",
                    name=f"attn_full{i}",
                )
                for i in range(NTT)
            ]

            def proj(ti):
                tsl = slice(TT * ti, TT * (ti + 1))
                q_ps = [
                    ps.tile([128, TT], _F32, tag=f"p{h}", name=f"q_ps{h}")
                    for h in range(HQ)
                ]
                k_ps = ps.tile([128, TT], _F32, tag="p4")
                vT_ps = ps.tile([128, TT], _F32, tag="p5")
                for hi in range(NH):
                    hsl = slice(128 * hi, 128 * (hi + 1))
                    if ti == 0:
                        # weight slice DMAs interleaved with x so the first
                        # matmuls aren't queued behind the whole preload
                        nc.sync.dma_start(out=wq_t[hi][:], in_=wqT[hsl, :])
                        nc.sync.dma_start(out=wk_t[hi][:], in_=wkT[hsl, :])
                        nc.sync.dma_start(out=wv_t[hi][:], in_=wvT[hsl, :])
                    xt = sb.tile([128, TT], _DT, tag="x", bufs=6)
                    nc.sync.dma_start(out=xt[:], in_=xT[hsl, tsl])
                    st, sp = hi == 0, hi == NH - 1
                    for h in range(HQ):
                        nc.tensor.matmul(
                            q_ps[h][:],
                            wq_t[hi][:, 128 * h : 128 * (h + 1)],
                            xt[:],
                            start=st,
                            stop=sp,
                        )
                    nc.tensor.matmul(k_ps[:], wk_t[hi][:], xt[:], start=st, stop=sp)
                    nc.tensor.matmul(vT_ps[:], wv_t[hi][:], xt[:], start=st, stop=sp)

                # V: transpose [d, s] -> [s, d] blocks
                vT_sb = sb.tile([128, TT], _DT, tag="vTs", bufs=2)
                nc.vector.tensor_copy(vT_sb[:], vT_ps[:])
                for j in range(TT // 128):
                    vtr = ps.tile([128, 128], _DT, tag="p6", bufs=2)
                    nc.tensor.transpose(
                        vtr[:], vT_sb[:, 128 * j : 128 * (j + 1)], ident_sb[:]
                    )
                    k = (TT // 128) * ti + j
                    nc.vector.tensor_copy(v_sb[:, 128 * k : 128 * (k + 1)], vtr[:])

                # RoPE on q heads and k (fp32 math, bf16 store)
                for h in range(HQ + 1):
                    src = q_ps[h] if h < HQ else k_ps
                    qf = sb.tile([128, TT], _F32, tag="qf", bufs=2)
                    nc.vector.tensor_copy(qf[:], src[:])
                    qs = sb.tile([128, TT], _F32, tag="qs", bufs=2)
                    nc.scalar.dma_start(out=qs[0:64, :], in_=qf[64:128, :])
                    nc.scalar.dma_start(out=qs[64:128, :], in_=qf[0:64, :])
                    t1 = sb.tile([128, TT], _F32, tag="t1", bufs=2)
                    nc.vector.tensor_mul(t1[:], qf[:], cos_sb[:, tsl])
                    t2 = sb.tile([128, TT], _F32, tag="t2", bufs=2)
                    nc.vector.tensor_mul(t2[:], qs[:], sin_sb[:, tsl])
                    dst = qrot[h][:, tsl] if h < HQ else krot[:, tsl]
                    nc.vector.tensor_add(dst, t1[:], t2[:])

            def attn(ti):
                tsl = slice(TT * ti, TT * (ti + 1))
                nblk = (TT // 128) * (ti + 1)
                for h in range(HQ):
                    attn_ps = ps.tile([128, TT], _F32, tag=f"p{h}")
                    den_ps = ps.tile([128, TT], _F32, tag="p4" if h % 2 == 0 else "p5")
                    for k in range(nblk):
                        ksl = slice(128 * k, 128 * (k + 1))
                        st, sp = k == 0, k == nblk - 1
                        diag = k - (TT // 128) * ti
                        # causal: block k only reaches t' >= 128*diag
                        lo = 128 * diag if diag > 0 else 0
                        qsl = slice(TT * ti + lo, TT * (ti + 1))
                        sc = ps.tile([128, TT], _F32, tag="p6", bufs=2)
                        nc.tensor.matmul(
                            sc[:, lo:TT],
                            krot[:, ksl],
                            qrot[h][:, qsl],
                            start=True,
                            stop=True,
                        )
                        probs = sb.tile([128, TT], _DT, tag="probs", bufs=3)
                        if diag >= 0:
                            ptmp = sb.tile([128, TT], _F32, tag="ptmp", bufs=2)
                            nc.scalar.activation(
                                ptmp[:, lo:TT], sc[:, lo:TT], Exp, scale=SCALE
                            )
                            nc.vector.tensor_mul(
                                probs[:, lo:TT],
                                ptmp[:, lo:TT],
                                mask_sb[:, TT * diag + lo : TT * (diag + 1)],
                            )
                        else:
                            nc.scalar.activation(probs[:], sc[:], Exp, scale=SCALE)
                        nc.tensor.matmul(
                            attn_ps[:, lo:TT],
                            v_sb[:, ksl],
                            probs[:, lo:TT],
                            start=st,
                            stop=sp,
                        )
                        nc.tensor.matmul(
                            den_ps[:, lo:TT],
                            ones_sb[:],
                            probs[:, lo:TT],
                            start=st,
                            stop=sp,
                        )
                    recip = sb.tile([128, TT], _F32, tag="recip", bufs=2)
                    nc.vector.reciprocal_approx_fast(recip[:], den_ps[:])
                    anorm = sb.tile([128, TT], _DT, tag="anorm", bufs=2)
                    nc.vector.tensor_mul(anorm[:], attn_ps[:], recip[:])
                    nc.gpsimd.dma_start(
                        out=attn_local[ti][128 * h : 128 * (h + 1), :], in_=anorm[:]
                    )

            def gather(ti):
                nc.gpsimd.collective_compute(
                    "AllGather",
                    mybir.AluOpType.bypass,
                    replica_groups=[list(range(N_CORES))],
                    ins=[attn_local[ti].opt()],
                    outs=[attn_full[ti].opt()],
                )

            def outproj(ti, tags):
                o_ps = [
                    ps.tile(
                        [128, TT],
                        _F32,
                        tag=tg,
                        name=f"o_ps{ti}_{i}",
                        bufs=2 if tg == "p6" else 1,
                    )
                    for i, tg in enumerate(tags)
                ]
                for hd in range(NH):
                    ag = sb.tile([128, TT], _DT, tag="ag", bufs=6)
                    nc.gpsimd.dma_start(
                        out=ag[:], in_=attn_full[ti][128 * hd : 128 * (hd + 1), :]
                    )
                    st, sp = hd == 0, hd == NH - 1
                    for o in range(4):
                        nc.tensor.matmul(
                            o_ps[o][:],
                            wo_sb[:, 512 * hd + 128 * o : 512 * hd + 128 * (o + 1)],
                            ag[:],
                            start=st,
                            stop=sp,
                        )
                for o in range(4):
                    oc = sb.tile([128, TT], _F32, tag="oc", bufs=4)
                    nc.vector.tensor_copy(oc[:], o_ps[o][:])
                    nc.sync.dma_start(
                        out=out[128 * o : 128 * (o + 1), TT * ti : TT * (ti + 1)],
                        in_=oc[:],
                    )

            # pipeline: AG(ti) overlaps attn(ti+1); outproj(ti) follows
            proj(0)
            attn(0)
            gather(0)
            proj(1)
            attn(1)
            gather(1)
            outproj(0, ["p0", "p1", "p2", "p3"])
            proj(2)
            attn(2)
            gather(2)
            outproj(1, ["p0", "p1", "p2", "p3"])
            proj(3)
            attn(3)
            gather(3)
            outproj(2, ["p0", "p1", "p2", "p3"])
            outproj(3, ["p4", "p5", "p6", "p6"])

    nc.compile()
    return nc


def _host_inputs(hidden_states, Wq, Wk, Wv, Wo):
    import ml_dtypes

    bf16 = ml_dtypes.bfloat16
    x = np.asarray(hidden_states, dtype=np.float32).reshape(T, HID)
    xT = np.ascontiguousarray(x.T).astype(bf16)

    pos = np.arange(T, dtype=np.float32)
    inv_freq = ROPE_BASE ** (-np.arange(0, D, 2, dtype=np.float32) / D)  # [64]
    ang = pos[:, None] * inv_freq[None, :]  # [T, 64]
    cosT = np.cos(ang).T.astype(np.float32)  # [64, T]
    sinT = np.sin(ang).T.astype(np.float32)
    cos2 = np.ascontiguousarray(np.concatenate([cosT, cosT], axis=0))
    sinS = np.ascontiguousarray(np.concatenate([-sinT, sinT], axis=0))

    p = np.arange(128)[:, None]
    tp = np.arange(TT)[None, :]
    masks = np.concatenate(
        [(p <= tp - 128 * j).astype(np.float32) for j in range(4)], axis=1
    )
    masks = np.ascontiguousarray(masks)
    ones = np.ones((128, 128), dtype=bf16)
    ident = np.eye(128, dtype=np.float32).astype(bf16)

    Wq = np.asarray(Wq, dtype=np.float32)
    Wk = np.asarray(Wk, dtype=np.float32)
    Wv = np.asarray(Wv, dtype=np.float32)
    Wo = np.asarray(Wo, dtype=np.float32)

    in_maps = []
    for c in range(N_CORES):
        qs = slice(HQ * D * c, HQ * D * (c + 1))
        ks = slice(D * c, D * (c + 1))
        in_maps.append(
            {
                "xT": xT,
                "wqT": np.ascontiguousarray(Wq[qs, :].T).astype(bf16),
                "wkT": np.ascontiguousarray(Wk[ks, :].T).astype(bf16),
                "wvT": np.ascontiguousarray(Wv[ks, :].T).astype(bf16),
                "woT": np.ascontiguousarray(Wo[qs, :].T).astype(bf16),
                "cos2": cos2,
                "sinS": sinS,
                "masks": masks,
                "ones_i": ones,
                "ident_i": ident,
            }
        )
    return in_maps


def get_program():
    global _cached
    if _cached is None:
        _cached = _build()
    return _cached


def kernel(hidden_states, Wq, Wk, Wv, Wo):
    nc = get_program()
    in_maps = _host_inputs(hidden_states, Wq, Wk, Wv, Wo)
    res = run_bass_kernel_spmd(nc, in_maps, list(range(N_CORES)))
    outT = np.concatenate([res.results[c]["out"] for c in range(N_CORES)], axis=0)
    return np.ascontiguousarray(outT.T).reshape(1, T, HID).astype(np.float32)


# revision 7
# speedup vs baseline: 1.5943x; 1.0229x over previous
"""Trainium2 Bass kernel for GQA attention layer (B=1, T=2048, HID=4096,
32 q-heads / 8 kv-heads, head_dim 128, RoPE, causal) sharded over 8 cores.

Sharding: tensor-parallel over heads. Core c owns q-heads 4c..4c+3 and
kv-head c. Attention outputs (transposed, [512 hd, t]) are AllGathered in
four t-chunks (pipelined against later attention compute); each core then
computes a 512-row slice of the output projection over the full 4096 hd
dims, so no AllReduce is needed. Host assembles the 8 output slices.

Matmuls run in bf16 (PE moving operand streams 2B/cycle, so bf16 is 2x
fp32r), accumulation in fp32 PSUM; softmax statistics in fp32.
"""

import numpy as np

import concourse.bacc as bacc
import concourse.mybir as mybir
import concourse.tile as tile
from concourse.bass_utils import run_bass_kernel_spmd

T = 2048
HID = 4096
D = 128
N_HEADS = 32
N_KV = 8
HQ = N_HEADS // N_KV  # q heads per core (=4)
TT = 512  # t tile
NTT = T // TT  # 4
NH = HID // 128  # 32 h-tiles
SCALE = 1.0 / np.sqrt(np.float32(D))
ROPE_BASE = 10000.0
N_CORES = 8

_F32 = mybir.dt.float32
_DT = mybir.dt.bfloat16

_cached = None


def _build():
    nc = bacc.Bacc("TRN2", target_bir_lowering=False, debug=False, num_devices=N_CORES)

    xT = nc.dram_tensor("xT", [HID, T], _DT, kind="ExternalInput").ap()
    wqT = nc.dram_tensor("wqT", [HID, HQ * D], _DT, kind="ExternalInput").ap()
    wkT = nc.dram_tensor("wkT", [HID, D], _DT, kind="ExternalInput").ap()
    wvT = nc.dram_tensor("wvT", [HID, D], _DT, kind="ExternalInput").ap()
    woT = nc.dram_tensor("woT", [HID, HQ * D], _DT, kind="ExternalInput").ap()
    cos2 = nc.dram_tensor("cos2", [128, T], _F32, kind="ExternalInput").ap()
    sinS = nc.dram_tensor("sinS", [128, T], _F32, kind="ExternalInput").ap()
    masks = nc.dram_tensor("masks", [128, 4 * TT], _F32, kind="ExternalInput").ap()
    ones_i = nc.dram_tensor("ones_i", [128, 128], _DT, kind="ExternalInput").ap()
    ident_i = nc.dram_tensor("ident_i", [128, 128], _DT, kind="ExternalInput").ap()
    out = nc.dram_tensor("out", [HQ * D, T], _F32, kind="ExternalOutput").ap()

    Exp = mybir.ActivationFunctionType.Exp

    with tile.TileContext(nc) as tc:
        with (
            tc.tile_pool(name="const", bufs=1) as const,
            tc.tile_pool(name="big", bufs=1) as big,
            tc.tile_pool(name="sb", bufs=1) as sb,
            tc.tile_pool(name="ps", bufs=1, space="PSUM") as ps,
            tc.tile_pool(name="dram", bufs=1, space="DRAM") as dram,
        ):
            # ---- constants / persistent weights in SBUF ----
            cos_sb = const.tile([128, T], _F32, name="cos_sb")
            sin_sb = const.tile([128, T], _F32, name="sin_sb")
            mask_sb = const.tile([128, 4 * TT], _F32, name="mask_sb")
            ones_sb = const.tile([128, 128], _DT, name="ones_sb")
            ident_sb = const.tile([128, 128], _DT, name="ident_sb")
            # per-h-tile weight tiles so the first matmuls only wait on
            # their own slice's DMA, not the whole preload
            wq_t = [const.tile([128, HQ * D], _DT, name=f"wq_t{j}") for j in range(NH)]
            wk_t = [const.tile([128, D], _DT, name=f"wk_t{j}") for j in range(NH)]
            wv_t = [const.tile([128, D], _DT, name=f"wv_t{j}") for j in range(NH)]
            wo_sb = const.tile([128, NH * HQ * D], _DT, name="wo_sb")
            for j in range(NH):
                hsl = slice(128 * j, 128 * (j + 1))
                nc.gpsimd.dma_start(
                    out=wo_sb[:, 512 * j : 512 * (j + 1)], in_=woT[hsl, :]
                )

            nc.scalar.dma_start(out=cos_sb[:], in_=cos2[:])
            nc.scalar.dma_start(out=sin_sb[:], in_=sinS[:])
            nc.scalar.dma_start(out=mask_sb[:], in_=masks[:])
            nc.scalar.dma_start(out=ones_sb[:], in_=ones_i[:])
            nc.scalar.dma_start(out=ident_sb[:], in_=ident_i[:])

            qrot = [big.tile([128, T], _DT, name=f"qrot{h}") for h in range(HQ)]
            krot = big.tile([128, T], _DT, name="krot")
            v_sb = big.tile([128, T], _DT, name="v_sb")  # V[s,d]: block k at cols 128k

            attn_local = [
                dram.tile([HQ * D, TT], _DT, name=f"attn_local{i}") for i in range(NTT)
            ]
            attn_full = [
                dram.tile(
                    [N_CORES * HQ * D, TT],
                    _DT,
                    addr_space="Shared",
                    name=f"attn_full{i}",
                )
                for i in range(NTT)
            ]

            def proj(ti):
                tsl = slice(TT * ti, TT * (ti + 1))
                q_ps = [
                    ps.tile([128, TT], _F32, tag=f"p{h}", name=f"q_ps{h}")
                    for h in range(HQ)
                ]
                k_ps = ps.tile([128, TT], _F32, tag="p4")
                vT_ps = ps.tile([128, TT], _F32, tag="p5")
                for hi in range(NH):
                    hsl = slice(128 * hi, 128 * (hi + 1))
                    if ti == 0:
                        # weight slice DMAs interleaved with x so the first
                        # matmuls aren't queued behind the whole preload
                        nc.sync.dma_start(out=wq_t[hi][:], in_=wqT[hsl, :])
                        nc.sync.dma_start(out=wk_t[hi][:], in_=wkT[hsl, :])
                        nc.sync.dma_start(out=wv_t[hi][:], in_=wvT[hsl, :])
                    xt = sb.tile([128, TT], _DT, tag="x", bufs=12)
                    nc.sync.dma_start(out=xt[:], in_=xT[hsl, tsl])
                    st, sp = hi == 0, hi == NH - 1
                    for h in range(HQ):
                        nc.tensor.matmul(
                            q_ps[h][:],
                            wq_t[hi][:, 128 * h : 128 * (h + 1)],
                            xt[:],
                            start=st,
                            stop=sp,
                        )
                    nc.tensor.matmul(k_ps[:], wk_t[hi][:], xt[:], start=st, stop=sp)
                    nc.tensor.matmul(vT_ps[:], wv_t[hi][:], xt[:], start=st, stop=sp)

                # V: transpose [d, s] -> [s, d] blocks
                vT_sb = sb.tile([128, TT], _DT, tag="vTs", bufs=2)
                nc.vector.tensor_copy(vT_sb[:], vT_ps[:])
                for j in range(TT // 128):
                    vtr = ps.tile([128, 128], _DT, tag="p6", bufs=2)
                    nc.tensor.transpose(
                        vtr[:], vT_sb[:, 128 * j : 128 * (j + 1)], ident_sb[:]
                    )
                    k = (TT // 128) * ti + j
                    nc.vector.tensor_copy(v_sb[:, 128 * k : 128 * (k + 1)], vtr[:])

                # RoPE on q heads and k (fp32 math, bf16 store)
                for h in [0, HQ, 1, 2, 3]:
                    src = q_ps[h] if h < HQ else k_ps
                    qf = sb.tile([128, TT], _F32, tag="qf", bufs=2)
                    nc.vector.tensor_copy(qf[:], src[:])
                    qs = sb.tile([128, TT], _F32, tag="qs", bufs=2)
                    nc.scalar.dma_start(out=qs[0:64, :], in_=qf[64:128, :])
                    nc.scalar.dma_start(out=qs[64:128, :], in_=qf[0:64, :])
                    t1 = sb.tile([128, TT], _F32, tag="t1", bufs=2)
                    nc.vector.tensor_mul(t1[:], qf[:], cos_sb[:, tsl])
                    t2 = sb.tile([128, TT], _F32, tag="t2", bufs=2)
                    nc.vector.tensor_mul(t2[:], qs[:], sin_sb[:, tsl])
                    dst = qrot[h][:, tsl] if h < HQ else krot[:, tsl]
                    nc.vector.tensor_add(dst, t1[:], t2[:])

            def attn(ti):
                nblk = (TT // 128) * (ti + 1)
                for h in range(HQ):
                    attn_ps = ps.tile([128, TT], _F32, tag=f"p{h}")
                    den_ps = ps.tile([128, TT], _F32, tag="p4" if h % 2 == 0 else "p5")
                    sc_tags = [f"p{(h + 1 + j) % HQ}" for j in range(3)]
                    sc_t = {}
                    probs_t = {}

                    def lo_of(k):
                        diag = k - (TT // 128) * ti
                        return 128 * diag if diag > 0 else 0

                    def emit_sc(k):
                        # scoresT block + exp (ACT); causal sub-range only
                        lo = lo_of(k)
                        diag = k - (TT // 128) * ti
                        qsl = slice(TT * ti + lo, TT * (ti + 1))
                        sc = ps.tile(
                            [128, TT], _F32, tag=sc_tags[k % 3], name=f"sc{k}"
                        )
                        nc.tensor.matmul(
                            sc[:, lo:TT],
                            krot[:, 128 * k : 128 * (k + 1)],
                            qrot[h][:, qsl],
                            start=True,
                            stop=True,
                        )
                        probs = sb.tile([128, TT], _DT, tag="probs", bufs=5)
                        if diag >= 0:
                            ptmp = sb.tile([128, TT], _F32, tag="ptmp", bufs=3)
                            nc.scalar.activation(
                                ptmp[:, lo:TT], sc[:, lo:TT], Exp, scale=SCALE
                            )
                            nc.vector.tensor_mul(
                                probs[:, lo:TT],
                                ptmp[:, lo:TT],
                                mask_sb[:, TT * diag + lo : TT * (diag + 1)],
                            )
                        else:
                            nc.scalar.activation(probs[:], sc[:], Exp, scale=SCALE)
                        probs_t[k] = probs

                    for j in range(min(3, nblk)):
                        emit_sc(j)
                    for k in range(nblk):
                        if k + 3 < nblk:
                            emit_sc(k + 3)
                        lo = lo_of(k)
                        st, sp = k == 0, k == nblk - 1
                        probs = probs_t.pop(k)
                        nc.tensor.matmul(
                            attn_ps[:, lo:TT],
                            v_sb[:, 128 * k : 128 * (k + 1)],
                            probs[:, lo:TT],
                            start=st,
                            stop=sp,
                        )
                        nc.tensor.matmul(
                            den_ps[:, lo:TT],
                            ones_sb[:],
                            probs[:, lo:TT],
                            start=st,
                            stop=sp,
                        )
                    recip = sb.tile([128, TT], _F32, tag="recip", bufs=2)
                    nc.vector.reciprocal_approx_fast(recip[:], den_ps[:])
                    anorm = sb.tile([128, TT], _DT, tag="anorm", bufs=2)
                    nc.vector.tensor_mul(anorm[:], attn_ps[:], recip[:])
                    nc.gpsimd.dma_start(
                        out=attn_local[ti][128 * h : 128 * (h + 1), :], in_=anorm[:]
                    )

            def gather(ti):
                nc.gpsimd.collective_compute(
                    "AllGather",
                    mybir.AluOpType.bypass,
                    replica_groups=[list(range(N_CORES))],
                    ins=[attn_local[ti].opt()],
                    outs=[attn_full[ti].opt()],
                )

            def outproj(ti, tags):
                o_ps = [
                    ps.tile(
                        [128, TT],
                        _F32,
                        tag=tg,
                        name=f"o_ps{ti}_{i}",
                        bufs=2 if tg == "p6" else 1,
                    )
                    for i, tg in enumerate(tags)
                ]
                for hd in range(NH):
                    ag = sb.tile([128, TT], _DT, tag="ag", bufs=8)
                    nc.gpsimd.dma_start(
                        out=ag[:], in_=attn_full[ti][128 * hd : 128 * (hd + 1), :]
                    )
                    st, sp = hd == 0, hd == NH - 1
                    for o in range(4):
                        nc.tensor.matmul(
                            o_ps[o][:],
                            wo_sb[:, 512 * hd + 128 * o : 512 * hd + 128 * (o + 1)],
                            ag[:],
                            start=st,
                            stop=sp,
                        )
                for o in range(4):
                    oc = sb.tile([128, TT], _F32, tag="oc", bufs=4)
                    nc.vector.tensor_copy(oc[:], o_ps[o][:])
                    nc.sync.dma_start(
                        out=out[128 * o : 128 * (o + 1), TT * ti : TT * (ti + 1)],
                        in_=oc[:],
                    )

            # pipeline: AG(ti) overlaps attn(ti+1); outproj(ti) follows
            proj(0)
            attn(0)
            gather(0)
            proj(1)
            attn(1)
            gather(1)
            outproj(0, ["p0", "p1", "p2", "p3"])
            proj(2)
            attn(2)
            gather(2)
            outproj(1, ["p0", "p1", "p2", "p3"])
            proj(3)
            attn(3)
            gather(3)
            outproj(2, ["p0", "p1", "p2", "p3"])
            outproj(3, ["p4", "p5", "p6", "p6"])

    nc.compile()
    return nc


def _host_inputs(hidden_states, Wq, Wk, Wv, Wo):
    import ml_dtypes

    bf16 = ml_dtypes.bfloat16
    x = np.asarray(hidden_states, dtype=np.float32).reshape(T, HID)
    xT = np.ascontiguousarray(x.T).astype(bf16)

    pos = np.arange(T, dtype=np.float32)
    inv_freq = ROPE_BASE ** (-np.arange(0, D, 2, dtype=np.float32) / D)  # [64]
    ang = pos[:, None] * inv_freq[None, :]  # [T, 64]
    cosT = np.cos(ang).T.astype(np.float32)  # [64, T]
    sinT = np.sin(ang).T.astype(np.float32)
    cos2 = np.ascontiguousarray(np.concatenate([cosT, cosT], axis=0))
    sinS = np.ascontiguousarray(np.concatenate([-sinT, sinT], axis=0))

    p = np.arange(128)[:, None]
    tp = np.arange(TT)[None, :]
    masks = np.concatenate(
        [(p <= tp - 128 * j).astype(np.float32) for j in range(4)], axis=1
    )
    masks = np.ascontiguousarray(masks)
    ones = np.ones((128, 128), dtype=bf16)
    ident = np.eye(128, dtype=np.float32).astype(bf16)

    Wq = np.asarray(Wq, dtype=np.float32)
    Wk = np.asarray(Wk, dtype=np.float32)
    Wv = np.asarray(Wv, dtype=np.float32)
    Wo = np.asarray(Wo, dtype=np.float32)

    in_maps = []
    for c in range(N_CORES):
        qs = slice(HQ * D * c, HQ * D * (c + 1))
        ks = slice(D * c, D * (c + 1))
        in_maps.append(
            {
                "xT": xT,
                "wqT": np.ascontiguousarray(Wq[qs, :].T).astype(bf16),
                "wkT": np.ascontiguousarray(Wk[ks, :].T).astype(bf16),
                "wvT": np.ascontiguousarray(Wv[ks, :].T).astype(bf16),
                "woT": np.ascontiguousarray(Wo[qs, :].T).astype(bf16),
                "cos2": cos2,
                "sinS": sinS,
                "masks": masks,
                "ones_i": ones,
                "ident_i": ident,
            }
        )
    return in_maps


def get_program():
    global _cached
    if _cached is None:
        _cached = _build()
    return _cached


def kernel(hidden_states, Wq, Wk, Wv, Wo):
    nc = get_program()
    in_maps = _host_inputs(hidden_states, Wq, Wk, Wv, Wo)
    res = run_bass_kernel_spmd(nc, in_maps, list(range(N_CORES)))
    outT = np.concatenate([res.results[c]["out"] for c in range(N_CORES)], axis=0)
    return np.ascontiguousarray(outT.T).reshape(1, T, HID).astype(np.float32)
